# revision 1
# baseline (speedup 1.0000x reference)
"""GRU decoder kernel for Trainium2 (Bass/Tile), data-parallel over 8 cores.

Problem: nn_Decoder (B=512, T=128, D=256, H=1024), PyTorch GRUCell:
    gi = x @ W_ih.T + b_ih ; gh = h @ W_hh.T + b_hh
    r = sig(gi_r+gh_r); z = sig(gi_z+gh_z); n = tanh(gi_n + r*gh_n)
    h' = (1-z)*n + z*h ; y = x + h' @ W_tp.T + b_tp ; x' = y (x0=gt[:,0,:])

Design (batch 64/core, weights replicated; ~12us/step, ~1.5ms total,
rel_err ~5e-3 vs fp32 reference):
  - "Folded-H" layout: gate tiles [128, 512] fp32 PSUM where partitions
    0:64 = (batch, gate cols 0:512) and 64:128 = (batch, cols 512:1024).
    Both column groups are produced CONCURRENTLY by fp16 matmul pairs at
    tile_position (0,0)/(0,64) -> 2x PE throughput vs one 64-row group
    (measured 216ns per N=512 pair slot).  fp32/f32r cannot use col_grp
    != 0 (invalid ISA); fp16 (10-bit mantissa) keeps 128-step error ~8x
    below bf16.
  - r/hn/in use full-width N=512 slots (2x53ns LDW hides under the 213ns
    stream); z is split into two 256-wide half-groups in SEPARATE PSUM
    banks so sig(z)->u->h' pipelines per half.  (A PSUM bank must never
    be engine-read while the PE streams into another region of the same
    bank - that corrupts the read.)
  - Elementwise chain is split into halves and spread over ACT/DVE so the
    h' tail overlaps the other half's matmuls.
  - Bias rows for step t+1 are pre-issued (start=True) into the PE wait
    for h'; the y head of step t is emitted early in iteration t+1 to
    overlap next-step gate matmuls.  This also avoids >1us PE idle gaps
    that would re-throttle the PE clock (HAM) to 1.2 GHz.
  - h' transposes: PE transpose-mode pairs, row groups 0/64 concurrently;
    the two row groups MUST write different PSUM banks (same-bank pair
    crashes the device).  A-half hT copies run inside the h'B window.
"""
import numpy as np

B, T, D, H = 512, 128, 256, 1024
NCORES = 8
BL = B // NCORES  # 64
H3 = 3 * H
KH = H // 128  # 8
KD = D // 128  # 2
HB = 512       # folded column width (H/2)
HQ = 256       # z half width

_CACHE = {}


def _build(nsteps):
    import concourse.bass as bass
    import concourse.mybir as mybir
    import concourse.tile as tile
    from concourse import bacc
    from concourse.masks import make_identity

    F32 = mybir.dt.float32
    F32R = mybir.dt.float32r
    BF16 = mybir.dt.float16  # fp16: 10-bit mantissa, col-tiling legal
    AF = mybir.ActivationFunctionType

    nc = bacc.Bacc(None, target_bir_lowering=False)

    hf0_d = nc.dram_tensor("hf0", [128, HB], F32, kind="ExternalInput")
    hT0_d = nc.dram_tensor("hT0", [128, KH, BL], BF16, kind="ExternalInput")
    xf0_d = nc.dram_tensor("xf0", [128, D // 2], F32, kind="ExternalInput")
    xT0_d = nc.dram_tensor("xT0", [128, KD, BL], BF16, kind="ExternalInput")
    whh_d = nc.dram_tensor("whhT", [H, H3], BF16, kind="ExternalInput")
    wih_d = nc.dram_tensor("wihT", [D, H3], BF16, kind="ExternalInput")
    wtp_d = nc.dram_tensor("wtpT", [H, D], BF16, kind="ExternalInput")
    brz_d = nc.dram_tensor("brz", [1, 2 * H], BF16, kind="ExternalInput")
    bni_d = nc.dram_tensor("bni", [1, 2 * H], BF16, kind="ExternalInput")
    btp_d = nc.dram_tensor("btp", [1, D], BF16, kind="ExternalInput")
    ones_d = nc.dram_tensor("ones", [1, 128], BF16, kind="ExternalInput")
    Y_d = nc.dram_tensor("Y", [T, 128, D // 2], F32, kind="ExternalOutput")

    with tile.TileContext(nc) as tc:
        with (
            tc.tile_pool(name="wpool", bufs=1) as wpool,
            tc.tile_pool(name="state", bufs=2) as state,
            tc.tile_pool(name="gates", bufs=1) as gates,
            tc.tile_pool(name="ypool", bufs=3) as ypool,
            tc.tile_pool(name="ps_r", bufs=1, space="PSUM") as ps_r,
            tc.tile_pool(name="ps_za", bufs=1, space="PSUM") as ps_za,
            tc.tile_pool(name="ps_zb", bufs=1, space="PSUM") as ps_zb,
            tc.tile_pool(name="ps_hn", bufs=1, space="PSUM") as ps_hn,
            tc.tile_pool(name="ps_in", bufs=1, space="PSUM") as ps_in,
            tc.tile_pool(name="ps_y", bufs=1, space="PSUM") as ps_y,
            tc.tile_pool(name="ps_ta", bufs=1, space="PSUM") as ps_ta,
            tc.tile_pool(name="ps_tb", bufs=1, space="PSUM") as ps_tb,
        ):
            # --- resident weights
            whh = wpool.tile([128, KH, H3], BF16)
            wih = wpool.tile([128, KD, H3], BF16)
            wtp = wpool.tile([128, KH, D], BF16)
            for c in range(KH):
                nc.sync.dma_start(out=whh[:, c, :], in_=whh_d[c * 128:(c + 1) * 128, :])
                nc.sync.dma_start(out=wtp[:, c, :], in_=wtp_d[c * 128:(c + 1) * 128, :])
            for c in range(KD):
                nc.sync.dma_start(out=wih[:, c, :], in_=wih_d[c * 128:(c + 1) * 128, :])
            brz = wpool.tile([1, 2 * H], BF16)
            bni = wpool.tile([1, 2 * H], BF16)
            btp = wpool.tile([1, D], BF16)
            ones = wpool.tile([1, 128], BF16)
            nc.sync.dma_start(out=brz, in_=brz_d[:, :])
            nc.sync.dma_start(out=bni, in_=bni_d[:, :])
            nc.sync.dma_start(out=btp, in_=btp_d[:, :])
            nc.sync.dma_start(out=ones, in_=ones_d[:, :])
            ident = wpool.tile([128, 64], F32)
            make_identity(nc, ident[0:64, :])
            make_identity(nc, ident[64:128, :])

            # --- state
            hf = state.tile([128, HB], F32, tag="hf")
            hT = state.tile([128, KH, BL], BF16, tag="hT")
            xn = state.tile([128, D // 2], F32, tag="xn")
            xT = state.tile([128, KD, BL], BF16, tag="xT")
            nc.sync.dma_start(out=hf, in_=hf0_d[:, :])
            nc.sync.dma_start(out=hT, in_=hT0_d[:, :, :])
            nc.sync.dma_start(out=xn, in_=xf0_d[:, :])
            nc.sync.dma_start(out=xT, in_=xT0_d[:, :, :])

            HORD = [0, 1, 4, 5, 2, 3, 6, 7]

            def pair(psum, cols, lhsT, rhs0, rhs1, start=False, stop=False):
                nc.tensor.matmul(psum[0:64, cols], lhsT, rhs0, start=start, stop=stop)
                nc.tensor.matmul(psum[64:128, cols], lhsT, rhs1, start=start, stop=stop)

            def bias_pair(psum, cols, bias_ap, b0, b1, width, start, stop):
                nc.tensor.matmul(psum[0:64, cols], ones[:, 0:64],
                                 bias_ap[:, b0:b0 + width], start=start, stop=stop)
                nc.tensor.matmul(psum[64:128, cols], ones[:, 64:128],
                                 bias_ap[:, b1:b1 + width], start=start, stop=stop)

            ALL = slice(0, HB)

            def emit_bias_next():
                """Fresh psum tiles for the next step; start each group with
                its bias row.  Returns the tiles."""
                p_r = ps_r.tile([128, HB], F32, tag="r")
                p_za = ps_za.tile([128, HQ], F32, tag="za")
                p_zb = ps_zb.tile([128, HQ], F32, tag="zb")
                p_hn = ps_hn.tile([128, HB], F32, tag="hn")
                p_in = ps_in.tile([128, HB], F32, tag="in")
                return p_r, (p_za, p_zb), p_hn, p_in

            def emit_bias_rhnin(p_r, p_hn, p_in):
                bias_pair(p_r, ALL, brz, 0, HB, HB, True, False)
                bias_pair(p_hn, ALL, bni, 0, HB, HB, True, False)
                bias_pair(p_in, ALL, bni, H, H + HB, HB, True, False)

            def emit_bias_z(p_z):
                for q, p_zq in ((0, p_z[0]), (1, p_z[1])):
                    bias_pair(p_zq, slice(0, HQ), brz, H + HQ * q,
                              H + HB + HQ * q, HQ, True, False)

            def emit_y_head(hT_t, x_t, t, p_y):
                """fp16 folded y head for step t: y = x + h'@W_tp.T + b_tp.
                Folded y [128, 128]: parts 0:64 = y cols 0:128 (group0),
                parts 64:128 = y cols 128:256 (group1)."""
                for c in (2, 3, 6, 7):
                    nc.tensor.matmul(p_y[0:64, :], hT_t[:, c, :],
                                     wtp[:, c, 0:128], start=False, stop=False)
                    nc.tensor.matmul(p_y[64:128, :], hT_t[:, c, :],
                                     wtp[:, c, 128:256], start=False, stop=False)
                nc.tensor.matmul(p_y[0:64, :], ones[:, 0:64], btp[:, 0:128],
                                 start=False, stop=True)
                nc.tensor.matmul(p_y[64:128, :], ones[:, 64:128], btp[:, 128:256],
                                 start=False, stop=True)
                y = ypool.tile([128, D // 2], F32, tag="y")
                nc.vector.tensor_add(y, x_t, p_y)
                nc.sync.dma_start(out=Y_d[t, :, :], in_=y)
                # xT for the gates of step t+1 (concurrent row pair)
                p_ta = ps_ta.tile([128, 5 * BL], F32, tag="ta")
                p_tb = ps_tb.tile([128, 5 * BL], F32, tag="tb")
                nc.tensor.transpose(p_ta[:, 4 * BL:5 * BL], y[0:64, :],
                                    ident[0:64, :])
                nc.tensor.transpose(p_tb[:, 4 * BL:5 * BL], y[64:128, :],
                                    ident[64:128, :])
                xT_new = state.tile([128, KD, BL], BF16, tag="xT")
                nc.vector.tensor_copy(xT_new[:, 0, :], p_ta[:, 4 * BL:5 * BL])
                nc.vector.tensor_copy(xT_new[:, 1, :], p_tb[:, 4 * BL:5 * BL])
                return y, xT_new, p_ta, p_tb

            # step 0 bias pre-issue
            p_r, p_z, p_hn, p_in = emit_bias_next()
            emit_bias_rhnin(p_r, p_hn, p_in)
            emit_bias_z(p_z)
            pend_y = None  # (hT_t, x_t, t) for the deferred y head

            for t in range(nsteps):
                # --- r h-chunk slots
                for c in HORD:
                    pair(p_r, ALL, hT[:, c, :], whh[:, c, 0:HB], whh[:, c, HB:H])
                # --- deferred y head of step t-1 (overlaps r matmuls)
                if pend_y is not None:
                    y, xT_new, p_ta, p_tb = emit_y_head(*pend_y)
                    xn, xT = y, xT_new
                # --- r x-chunk slots (need xT of this step) + stop
                pair(p_r, ALL, xT[:, 0, :], wih[:, 0, 0:HB], wih[:, 0, HB:H])
                pair(p_r, ALL, xT[:, 1, :], wih[:, 1, 0:HB], wih[:, 1, HB:H],
                     stop=True)
                # --- hn slots
                for i, c in enumerate(HORD):
                    pair(p_hn, ALL, hT[:, c, :], whh[:, c, 2 * H:2 * H + HB],
                         whh[:, c, 2 * H + HB:H3], stop=(i == KH - 1))
                # --- in slots
                pair(p_in, ALL, xT[:, 0, :], wih[:, 0, 2 * H:2 * H + HB],
                     wih[:, 0, 2 * H + HB:H3])
                pair(p_in, ALL, xT[:, 1, :], wih[:, 1, 2 * H:2 * H + HB],
                     wih[:, 1, 2 * H + HB:H3], stop=True)
                # --- z half-slots (separate banks per half: a bank being
                # matmul-written must not be concurrently read by ACT)
                for q, p_zq in ((0, p_z[0]), (1, p_z[1])):
                    hc = slice(0, HQ)
                    z0 = H + HQ * q
                    z1 = H + HB + HQ * q
                    for c in HORD:
                        pair(p_zq, hc, hT[:, c, :], whh[:, c, z0:z0 + HQ],
                             whh[:, c, z1:z1 + HQ])
                    pair(p_zq, hc, xT[:, 0, :], wih[:, 0, z0:z0 + HQ],
                         wih[:, 0, z1:z1 + HQ])
                    pair(p_zq, hc, xT[:, 1, :], wih[:, 1, z0:z0 + HQ],
                         wih[:, 1, z1:z1 + HQ], stop=True)

                # --- elementwise
                r_sb = gates.tile([128, HB], F32, tag="r")
                z_sb = gates.tile([128, HB], F32, tag="z")
                t1 = gates.tile([128, HB], F32, tag="t1")
                t2 = gates.tile([128, HB], F32, tag="t2")
                n_sb = gates.tile([128, HB], F32, tag="n")
                d_sb = gates.tile([128, HB], F32, tag="d")
                u_sb = gates.tile([128, HB], F32, tag="u")
                hf_new = state.tile([128, HB], F32, tag="hf")
                A = slice(0, HQ)
                Bc = slice(HQ, HB)
                # ACT: sig(rA), sig(rB), tanhA, sig(zA), tanhB, sig(zB)
                # DVE: t1A,t2A,t1B,t2B, dA,uA,h'A, dB,uB,h'B
                nc.scalar.activation(r_sb[:, A], p_r[:, A], AF.Sigmoid)
                nc.scalar.activation(r_sb[:, Bc], p_r[:, Bc], AF.Sigmoid)
                nc.vector.tensor_mul(t1[:, A], r_sb[:, A], p_hn[:, A])
                nc.vector.tensor_add(t2[:, A], t1[:, A], p_in[:, A])
                nc.vector.tensor_mul(t1[:, Bc], r_sb[:, Bc], p_hn[:, Bc])
                nc.vector.tensor_add(t2[:, Bc], t1[:, Bc], p_in[:, Bc])
                nc.scalar.activation(n_sb[:, A], t2[:, A], AF.Tanh)
                nc.scalar.activation(z_sb[:, A], p_z[0][:, :], AF.Sigmoid)
                nc.scalar.activation(n_sb[:, Bc], t2[:, Bc], AF.Tanh)
                nc.scalar.activation(z_sb[:, Bc], p_z[1][:, :], AF.Sigmoid)
                nc.vector.tensor_sub(d_sb[:, A], hf[:, A], n_sb[:, A])
                nc.vector.tensor_mul(u_sb[:, A], z_sb[:, A], d_sb[:, A])
                nc.vector.tensor_add(hf_new[:, A], n_sb[:, A], u_sb[:, A])
                nc.vector.tensor_sub(d_sb[:, Bc], hf[:, Bc], n_sb[:, Bc])
                nc.vector.tensor_mul(u_sb[:, Bc], z_sb[:, Bc], d_sb[:, Bc])
                nc.vector.tensor_add(hf_new[:, Bc], n_sb[:, Bc], u_sb[:, Bc])

                # --- transposes: pairs (c, c+4), banks ta/tb; first the two
                # chunks of folded half A (cols 0:256), then bias pre-issue
                # for t+1, then half B chunks
                p_ta = ps_ta.tile([128, 5 * BL], F32, tag="ta")
                p_tb = ps_tb.tile([128, 5 * BL], F32, tag="tb")
                hT_new = state.tile([128, KH, BL], BF16, tag="hT")
                # fill the PE wait for h'A with next-step bias rows
                if t + 1 < nsteps:
                    p_r, p_z, p_hn, p_in = emit_bias_next()
                    emit_bias_rhnin(p_r, p_hn, p_in)
                if t + 1 < nsteps:
                    emit_bias_z(p_z)
                for c in (0, 1):
                    nc.tensor.transpose(p_ta[:, c * BL:(c + 1) * BL],
                                        hf_new[0:64, c * 128:(c + 1) * 128],
                                        ident[0:64, :])
                    nc.tensor.transpose(p_tb[:, c * BL:(c + 1) * BL],
                                        hf_new[64:128, c * 128:(c + 1) * 128],
                                        ident[64:128, :])
                # A-half copies run inside the h'B window (h'B trails h'A by
                # the DVE chain, so these never overlap trB's bank writes)
                nc.scalar.copy(hT_new[:, 0:2, :], p_ta[:, 0:2 * BL])
                nc.scalar.copy(hT_new[:, 4:6, :], p_tb[:, 0:2 * BL])
                p_y = ps_y.tile([128, D // 2], F32, tag="y")
                for i, c in enumerate((0, 1, 4, 5)):
                    nc.tensor.matmul(p_y[0:64, :], hT_new[:, c, :],
                                     wtp[:, c, 0:128], start=(i == 0), stop=False)
                    nc.tensor.matmul(p_y[64:128, :], hT_new[:, c, :],
                                     wtp[:, c, 128:256], start=(i == 0), stop=False)
                for c in (2, 3):
                    nc.tensor.transpose(p_ta[:, c * BL:(c + 1) * BL],
                                        hf_new[0:64, c * 128:(c + 1) * 128],
                                        ident[0:64, :])
                    nc.tensor.transpose(p_tb[:, c * BL:(c + 1) * BL],
                                        hf_new[64:128, c * 128:(c + 1) * 128],
                                        ident[64:128, :])
                nc.scalar.copy(hT_new[:, 2:4, :], p_ta[:, 2 * BL:4 * BL])
                nc.vector.tensor_copy(hT_new[:, 6:8, :], p_tb[:, 2 * BL:4 * BL])

                pend_y = (hT_new, xn, t, p_y)
                hf, hT = hf_new, hT_new

            # flush the last y head
            emit_y_head(*pend_y)

    nc.finalize()
    return nc


def _build_null():
    import concourse.mybir as mybir
    import concourse.tile as tile
    from concourse import bacc

    F32 = mybir.dt.float32
    F32R = mybir.dt.float32r
    BF16 = mybir.dt.float16  # fp16: 10-bit mantissa, col-tiling legal
    nc = bacc.Bacc(None, target_bir_lowering=False)
    hf0_d = nc.dram_tensor("hf0", [128, HB], F32, kind="ExternalInput")
    nc.dram_tensor("hT0", [128, KH, BL], BF16, kind="ExternalInput")
    nc.dram_tensor("xf0", [128, D // 2], F32, kind="ExternalInput")
    nc.dram_tensor("xT0", [128, KD, BL], BF16, kind="ExternalInput")
    nc.dram_tensor("whhT", [H, H3], BF16, kind="ExternalInput")
    nc.dram_tensor("wihT", [D, H3], BF16, kind="ExternalInput")
    nc.dram_tensor("wtpT", [H, D], BF16, kind="ExternalInput")
    nc.dram_tensor("brz", [1, 2 * H], BF16, kind="ExternalInput")
    nc.dram_tensor("bni", [1, 2 * H], BF16, kind="ExternalInput")
    nc.dram_tensor("btp", [1, D], BF16, kind="ExternalInput")
    nc.dram_tensor("ones", [1, 128], BF16, kind="ExternalInput")
    Y_d = nc.dram_tensor("Y", [T, 128, D // 2], F32, kind="ExternalOutput")
    with tile.TileContext(nc) as tc:
        with tc.tile_pool(name="p", bufs=1) as p:
            tmp = p.tile([128, HB], F32)
            nc.sync.dma_start(out=tmp, in_=hf0_d[:, :])
            nc.sync.dma_start(out=Y_d[:, 0, :], in_=tmp[0:64, 0:256])
    nc.finalize()
    return nc


def _get_nc(nsteps):
    if nsteps not in _CACHE:
        _CACHE[nsteps] = _build(nsteps)
    return _CACHE[nsteps]


def _fold(a):
    w = a.shape[1] // 2
    return np.concatenate([a[:, :w], a[:, w:]], axis=0)


def make_in_maps(h, gt, W_ih, W_hh, b_ih, b_hh, W_tp, b_tp):
    f32 = np.float32
    h = np.asarray(h, f32)
    x0 = np.ascontiguousarray(np.asarray(gt, f32)[:, 0, :])
    whhT = np.ascontiguousarray(np.asarray(W_hh, f32).T).astype(np.float16)
    wihT = np.ascontiguousarray(np.asarray(W_ih, f32).T).astype(np.float16)
    wtpT = np.ascontiguousarray(np.asarray(W_tp, f32).T).astype(np.float16)
    b_sum = np.asarray(b_ih, f32) + np.asarray(b_hh, f32)
    brz = b_sum[None, :2 * H].astype(np.float16)
    bni = np.concatenate([np.asarray(b_hh, f32)[2 * H:],
                          np.asarray(b_ih, f32)[2 * H:]])[None, :].astype(np.float16)
    btp = np.ascontiguousarray(np.asarray(b_tp, f32)[None, :]).astype(np.float16)
    ones = np.ones((1, 128), np.float16)
    in_maps = []
    for c in range(NCORES):
        sl = slice(c * BL, (c + 1) * BL)
        hc = np.ascontiguousarray(h[sl])
        xc = np.ascontiguousarray(x0[sl])
        hTc = np.ascontiguousarray(hc.T.reshape(KH, 128, BL).transpose(1, 0, 2))
        xTc = np.ascontiguousarray(xc.T.reshape(KD, 128, BL).transpose(1, 0, 2))
        in_maps.append({
            "hf0": np.ascontiguousarray(_fold(hc)),
            "hT0": hTc.astype(np.float16),
            "xf0": np.ascontiguousarray(_fold(xc)),
            "xT0": xTc.astype(np.float16),
            "whhT": whhT, "wihT": wihT, "wtpT": wtpT,
            "brz": brz, "bni": bni, "btp": btp,
            "ones": ones,
        })
    return in_maps


def kernel(h, gt, W_ih, W_hh, b_ih, b_hh, W_tp, b_tp, time_steps):
    from concourse.bass_utils import run_bass_kernel_spmd
    nsteps = int(time_steps)
    assert nsteps == T, f"kernel hardcodes T={T}, got {nsteps}"
    nc = _get_nc(nsteps)
    in_maps = make_in_maps(h, gt, W_ih, W_hh, b_ih, b_hh, W_tp, b_tp)
    res = run_bass_kernel_spmd(nc, in_maps, core_ids=list(range(NCORES)),
                               trace=False)
    Y = np.concatenate([unfold_Y(res.results[c]["Y"]) for c in range(NCORES)],
                       axis=0)
    return Y.astype(np.float32)


def unfold_Y(Yd):
    """[T, 128, 128] -> [64, T, 256]"""
    out = np.empty((BL, T, D), np.float32)
    out[:, :, :D // 2] = Yd[:, 0:BL, :].transpose(1, 0, 2)
    out[:, :, D // 2:] = Yd[:, BL:128, :].transpose(1, 0, 2)
    return out



# revision 16
# speedup vs baseline: 9.4273x; 9.4273x over previous
"""GRU decoder kernel for Trainium2 (Bass/Tile), data-parallel over 8 cores.

Problem: nn_Decoder (B=512, T=128, D=256, H=1024), PyTorch GRUCell:
    gi = x @ W_ih.T + b_ih ; gh = h @ W_hh.T + b_hh
    r = sig(gi_r+gh_r); z = sig(gi_z+gh_z); n = tanh(gi_n + r*gh_n)
    h' = (1-z)*n + z*h ; y = x + h' @ W_tp.T + b_tp ; x' = y (x0=gt[:,0,:])

v4 design (batch 64/core, weights replicated):
  - "Folded-H" layout: gate tiles [128, N] fp32 PSUM; partitions 0:64 =
    (batch, gate col group 0), 64:128 = (batch, group 1), produced
    concurrently by fp16 matmul pairs at tile_position (0,0)/(0,64).
  - y-head FUSION: y_(t-1) = x_(t-1) + h_t@W_tp.T + b_tp rides as a
    128-col tail on the hn-A slots (N=384 = [W_hh_n A-cols | W_tp]).
    The hn bank gets no x-side writes, so DVE reads the y tail right
    after the 8 h-chunk slots with no same-bank write conflict.
  - SOFTWARE PIPELINE: step t+1's hnA slots run inside step t's endgame
    (between the h' transposes), so step t+1's r+rx complete early and
    the ACT/DVE gate chain overlaps the z streams.  PE order per step:
      r(1-2) yT r(3-8) rx inx hnB zA zB | T0 bias_r T1 bias(rest)
      hnA'(1-4) T2 T3 hnA'(5-8) | next step...
  - Elementwise in fp16 intermediates (2x DVE rate; bf16 fails the
    2e-2 gate, fp16 measures ~6e-3) quartered 128-col chains so the
    last quarter's sig->u->h' tail after zB is short.  d=h-n runs on
    the otherwise idle GPSIMD (SBUF-only operands).
  - Per-chunk PSUM->SBUF hT copies (ACT: chunks 0,4,1,5 / DVE: 2,6,3,7)
    unblock next-step hnA slots progressively.
  - Bias rows pre-issued into the PE h'-wait; z/hn half-groups in
    separate PSUM banks (a bank must not be engine-read while the PE
    streams into another region of the same bank).
"""
import numpy as np

B, T, D, H = 512, 128, 256, 1024
NCORES = 8
BL = B // NCORES  # 64
H3 = 3 * H
KH = H // 128  # 8
KD = D // 128  # 2
HB = 512       # folded column width (H/2)
HQ = 256       # half-group width
NA = HQ + 128  # hnA slot width (hn A-cols + y cols) = 384

_CACHE = {}


def _build(nsteps):
    import concourse.bass as bass
    import concourse.mybir as mybir
    import concourse.tile as tile
    from concourse import bacc
    from concourse.masks import make_identity

    F32 = mybir.dt.float32
    F16 = mybir.dt.float16
    AF = mybir.ActivationFunctionType

    nc = bacc.Bacc(None, target_bir_lowering=False)

    hf0_d = nc.dram_tensor("hf0", [128, HB], F16, kind="ExternalInput")
    hT0_d = nc.dram_tensor("hT0", [128, KH, BL], F16, kind="ExternalInput")
    xf0_d = nc.dram_tensor("xf0", [128, D // 2], F16, kind="ExternalInput")
    xT0_d = nc.dram_tensor("xT0", [128, KD, BL], F16, kind="ExternalInput")
    whh_d = nc.dram_tensor("whhT", [H, H3], F16, kind="ExternalInput")
    wih_d = nc.dram_tensor("wihT", [D, H3], F16, kind="ExternalInput")
    wna0_d = nc.dram_tensor("wna0", [H, NA], F16, kind="ExternalInput")
    wna1_d = nc.dram_tensor("wna1", [H, NA], F16, kind="ExternalInput")
    wrb0_d = nc.dram_tensor("wrb0", [D, HB], F16, kind="ExternalInput")
    wrb1_d = nc.dram_tensor("wrb1", [D, HB], F16, kind="ExternalInput")
    wtp_d = nc.dram_tensor("wtpT", [H, D], F16, kind="ExternalInput")
    brz_d = nc.dram_tensor("brz", [1, 2 * H], F16, kind="ExternalInput")
    brb_d = nc.dram_tensor("brb", [1, 2 * HB], F16, kind="ExternalInput")
    bna_d = nc.dram_tensor("bna", [1, 2 * NA], F16, kind="ExternalInput")
    bnb_d = nc.dram_tensor("bnb", [1, 2 * HB], F16, kind="ExternalInput")
    btp_d = nc.dram_tensor("btp", [1, D], F16, kind="ExternalInput")
    ones_d = nc.dram_tensor("ones", [1, 128], F16, kind="ExternalInput")
    Y_d = nc.dram_tensor("Y", [T, 128, D // 2], F16, kind="ExternalOutput")

    with tile.TileContext(nc) as tc:
        with (
            tc.tile_pool(name="wpool", bufs=1) as wpool,
            tc.tile_pool(name="state", bufs=2) as state,
            tc.tile_pool(name="gates", bufs=1) as gates,
            tc.tile_pool(name="ypool", bufs=3) as ypool,
            tc.tile_pool(name="ps_ra", bufs=1, space="PSUM") as ps_ra,
            tc.tile_pool(name="ps_na", bufs=1, space="PSUM") as ps_na,
            tc.tile_pool(name="ps_nb", bufs=1, space="PSUM") as ps_nb,
            tc.tile_pool(name="ps_rb", bufs=1, space="PSUM") as ps_rb,
            tc.tile_pool(name="ps_za", bufs=1, space="PSUM") as ps_za,
            tc.tile_pool(name="ps_zb", bufs=1, space="PSUM") as ps_zb,
            tc.tile_pool(name="ps_ta", bufs=1, space="PSUM") as ps_ta,
            tc.tile_pool(name="ps_tb", bufs=1, space="PSUM") as ps_tb,
        ):
            # --- resident weights
            whh = wpool.tile([128, KH, H3], F16)
            wih = wpool.tile([128, KD, H3], F16)
            wna0 = wpool.tile([128, KH, NA], F16)
            wna1 = wpool.tile([128, KH, NA], F16)
            wrb0 = wpool.tile([128, KD, HB], F16)
            wrb1 = wpool.tile([128, KD, HB], F16)
            wtp = wpool.tile([128, KH, D], F16)
            for c in range(KH):
                nc.sync.dma_start(out=whh[:, c, :], in_=whh_d[c * 128:(c + 1) * 128, :])
                nc.sync.dma_start(out=wna0[:, c, :], in_=wna0_d[c * 128:(c + 1) * 128, :])
                nc.sync.dma_start(out=wna1[:, c, :], in_=wna1_d[c * 128:(c + 1) * 128, :])
                nc.sync.dma_start(out=wtp[:, c, :], in_=wtp_d[c * 128:(c + 1) * 128, :])
            for c in range(KD):
                nc.sync.dma_start(out=wih[:, c, :], in_=wih_d[c * 128:(c + 1) * 128, :])
                nc.sync.dma_start(out=wrb0[:, c, :], in_=wrb0_d[c * 128:(c + 1) * 128, :])
                nc.sync.dma_start(out=wrb1[:, c, :], in_=wrb1_d[c * 128:(c + 1) * 128, :])
            brz = wpool.tile([1, 2 * H], F16)
            brb2 = wpool.tile([1, 2 * HB], F16)
            bna = wpool.tile([1, 2 * NA], F16)
            bnb2 = wpool.tile([1, 2 * HB], F16)
            btp = wpool.tile([1, D], F16)
            ones = wpool.tile([1, 128], F16)
            nc.sync.dma_start(out=brz, in_=brz_d[:, :])
            nc.sync.dma_start(out=brb2, in_=brb_d[:, :])
            nc.sync.dma_start(out=bna, in_=bna_d[:, :])
            nc.sync.dma_start(out=bnb2, in_=bnb_d[:, :])
            nc.sync.dma_start(out=btp, in_=btp_d[:, :])
            nc.sync.dma_start(out=ones, in_=ones_d[:, :])
            ident = wpool.tile([128, 64], F32)
            make_identity(nc, ident[0:64, :])
            make_identity(nc, ident[64:128, :])
            ident16 = wpool.tile([128, 64], F16)
            nc.vector.tensor_copy(ident16, ident)

            # --- state
            hf = state.tile([128, HB], F16, tag="hf")
            hT = state.tile([128, KH, BL], F16, tag="hT")
            xn = state.tile([128, D // 2], F16, tag="xn")
            xT = state.tile([128, KD, BL], F16, tag="xT")
            nc.sync.dma_start(out=hf, in_=hf0_d[:, :])
            nc.sync.dma_start(out=hT, in_=hT0_d[:, :, :])
            nc.sync.dma_start(out=xn, in_=xf0_d[:, :])
            nc.sync.dma_start(out=xT, in_=xT0_d[:, :, :])

            HORD = [0, 1, 4, 5, 2, 3, 6, 7]

            def pair(psum, cols, lhsT, rhs0, rhs1, start=False, stop=False):
                nc.tensor.matmul(psum[0:64, cols], lhsT, rhs0, start=start, stop=stop)
                nc.tensor.matmul(psum[64:128, cols], lhsT, rhs1, start=start, stop=stop)

            def bias_pair(psum, cols, bias_ap, b0, b1, width, start, stop):
                nc.tensor.matmul(psum[0:64, cols], ones[:, 0:64],
                                 bias_ap[:, b0:b0 + width], start=start, stop=stop)
                nc.tensor.matmul(psum[64:128, cols], ones[:, 64:128],
                                 bias_ap[:, b1:b1 + width], start=start, stop=stop)

            ALL = slice(0, HB)
            AQ = slice(0, HQ)
            NAs = slice(0, NA)

            def emit_tiles_next():
                p_ra = ps_ra.tile([128, HQ], F32, tag="ra")
                p_na = ps_na.tile([128, NA], F32, tag="na")
                p_nb = ps_nb.tile([128, HB], F32, tag="nb")
                p_rb = ps_rb.tile([128, HB], F32, tag="rb")
                p_za = ps_za.tile([128, HQ], F32, tag="za")
                p_zb = ps_zb.tile([128, HQ], F32, tag="zb")
                return p_ra, p_na, p_nb, p_rb, p_za, p_zb

            def emit_bias_all(cur):
                p_ra, p_na, p_nb, p_rb, p_za, p_zb = cur
                bias_pair(p_ra, AQ, brz, 0, HB, HQ, True, False)
                bias_pair(p_rb, ALL, brb2, 0, HB, HB, True, False)
                bias_pair(p_na, NAs, bna, 0, NA, NA, True, False)
                bias_pair(p_nb, ALL, bnb2, 0, HB, HB, True, False)
                bias_pair(p_za, AQ, brz, H, H + HB, HQ, True, False)
                bias_pair(p_zb, AQ, brz, H + HQ, H + HB + HQ, HQ, True, False)

            def emit_hna(p_na, hT_t, lo, hi):
                for i in range(lo, hi):
                    pair(p_na, NAs, hT_t[:, HORD[i], :], wna0[:, HORD[i], :],
                         wna1[:, HORD[i], :], stop=(i == KH - 1))

            # --- prologue: step-0 tiles, biases, hnA slots
            cur = emit_tiles_next()
            emit_bias_all(cur)
            emit_hna(cur[1], hT, 0, KH)

            for t in range(nsteps):
                p_ra, p_na, p_nb, p_rb, p_za, p_zb = cur
                # --- y_(t-1) extraction (DVE) and rA h-slots
                if t > 0:
                    y = ypool.tile([128, D // 2], F16, tag="y")
                    nc.vector.tensor_add(y, xn, p_na[:, HQ:NA])
                    nc.sync.dma_start(out=Y_d[t - 1, :, :], in_=y)
                for c in HORD:
                    pair(p_ra, AQ, hT[:, c, :], whh[:, c, 0:HQ],
                         whh[:, c, HB:HB + HQ])
                # --- yT transpose pair + xT copies (DVE)
                p_tay = ps_ta.tile([128, 5 * BL], F16, tag="ta")
                p_tby = ps_tb.tile([128, 5 * BL], F16, tag="tb")
                if t > 0:
                    nc.tensor.transpose(p_tay[:, 4 * BL:5 * BL], y[0:64, :],
                                        ident16[0:64, :])
                    nc.tensor.transpose(p_tby[:, 4 * BL:5 * BL], y[64:128, :],
                                        ident16[64:128, :])
                    xT_new = state.tile([128, KD, BL], F16, tag="xT")
                    nc.vector.tensor_copy(xT_new[:, 0, :], p_tay[:, 4 * BL:5 * BL])
                    nc.vector.tensor_copy(xT_new[:, 1, :], p_tby[:, 4 * BL:5 * BL])
                    xn, xT = y, xT_new
                # --- rB h-slots
                for c in HORD:
                    pair(p_rb, AQ, hT[:, c, :], whh[:, c, HQ:HB],
                         whh[:, c, HB + HQ:H])
                # --- rA x-slots (stop); combined rB+in x-slots (stop:
                # single final write per bank so every read dep covers all
                # bank writes)
                pair(p_ra, AQ, xT[:, 0, :], wih[:, 0, 0:HQ],
                     wih[:, 0, HB:HB + HQ])
                pair(p_ra, AQ, xT[:, 1, :], wih[:, 1, 0:HQ],
                     wih[:, 1, HB:HB + HQ], stop=True)
                pair(p_rb, ALL, xT[:, 0, :], wrb0[:, 0, :], wrb1[:, 0, :])
                pair(p_rb, ALL, xT[:, 1, :], wrb0[:, 1, :], wrb1[:, 1, :],
                     stop=True)
                # --- zA half-slots EARLY so sig(zA) -> u -> h'A
                # completes well before the zB stream ends
                def z_slots(p_zq, q):
                    z0 = H + HQ * q
                    z1 = H + HB + HQ * q
                    for c in HORD:
                        pair(p_zq, AQ, hT[:, c, :], whh[:, c, z0:z0 + HQ],
                             whh[:, c, z1:z1 + HQ])
                    pair(p_zq, AQ, xT[:, 0, :], wih[:, 0, z0:z0 + HQ],
                         wih[:, 0, z1:z1 + HQ])
                    pair(p_zq, AQ, xT[:, 1, :], wih[:, 1, z0:z0 + HQ],
                         wih[:, 1, z1:z1 + HQ], stop=True)

                # --- inB x-slots (nb-bank tail), then hnB h-slots: bank
                # fully written at the hnB stop, which gates t1B
                pair(p_nb, slice(HQ, HB), xT[:, 0, :],
                     wih[:, 0, 2 * H + HQ:2 * H + HB],
                     wih[:, 0, 2 * H + HB + HQ:H3])
                pair(p_nb, slice(HQ, HB), xT[:, 1, :],
                     wih[:, 1, 2 * H + HQ:2 * H + HB],
                     wih[:, 1, 2 * H + HB + HQ:H3], stop=True)
                for i, c in enumerate(HORD):
                    pair(p_nb, AQ, hT[:, c, :], whh[:, c, 2 * H + HQ:2 * H + HB],
                         whh[:, c, 2 * H + HB + HQ:H3], stop=(i == KH - 1))
                z_slots(p_za, 0)
                z_slots(p_zb, 1)

                # --- elementwise: fp16 intermediates; t1/sig/tanh halves,
                # t2 quartered (reads the three in-gate regions), d on
                # GPSIMD, u/h' quartered for a short post-zB tail.
                r_sb = gates.tile([128, HB], F16, tag="r")
                z_sb = gates.tile([128, HB], F16, tag="z")
                t1 = gates.tile([128, HB], F16, tag="t1")
                t2 = gates.tile([128, HB], F16, tag="t2")
                n_sb = gates.tile([128, HB], F16, tag="n")
                d_sb = gates.tile([128, HB], F16, tag="d")
                u_sb = gates.tile([128, HB], F16, tag="u")
                hf_new = state.tile([128, HB], F16, tag="hf")
                A = slice(0, HQ)
                Bc = slice(HQ, HB)
                Q = [slice(128 * q, 128 * (q + 1)) for q in range(4)]
                nc.scalar.activation(r_sb[:, A], p_ra[:, AQ], AF.Sigmoid)
                nc.scalar.activation(r_sb[:, Bc], p_rb[:, AQ], AF.Sigmoid)
                nc.vector.tensor_mul(t1[:, A], r_sb[:, A], p_na[:, AQ])
                nc.vector.tensor_mul(t1[:, Bc], r_sb[:, Bc], p_nb[:, AQ])
                nc.vector.tensor_add(t2[:, Q[0]], t1[:, Q[0]], p_rb[:, HQ:NA])
                nc.vector.tensor_add(t2[:, Q[1]], t1[:, Q[1]], p_rb[:, NA:HB])
                nc.vector.tensor_add(t2[:, Q[2]], t1[:, Q[2]], p_nb[:, HQ:NA])
                nc.vector.tensor_add(t2[:, Q[3]], t1[:, Q[3]], p_nb[:, NA:HB])
                nc.scalar.activation(n_sb[:, A], t2[:, A], AF.Tanh)
                nc.gpsimd.tensor_sub(d_sb[:, Q[0]], hf[:, Q[0]], n_sb[:, Q[0]])
                nc.gpsimd.tensor_sub(d_sb[:, Q[1]], hf[:, Q[1]], n_sb[:, Q[1]])
                nc.scalar.activation(z_sb[:, A], p_za[:, :], AF.Sigmoid)
                nc.scalar.activation(n_sb[:, Bc], t2[:, Bc], AF.Tanh)
                nc.gpsimd.tensor_sub(d_sb[:, Q[2]], hf[:, Q[2]], n_sb[:, Q[2]])
                nc.gpsimd.tensor_sub(d_sb[:, Q[3]], hf[:, Q[3]], n_sb[:, Q[3]])
                nc.scalar.activation(z_sb[:, Bc], p_zb[:, :], AF.Sigmoid)
                for q in range(4):
                    nc.vector.tensor_mul(u_sb[:, Q[q]], z_sb[:, Q[q]],
                                         d_sb[:, Q[q]])
                    nc.vector.tensor_add(hf_new[:, Q[q]], n_sb[:, Q[q]],
                                         u_sb[:, Q[q]])

                # --- endgame: T0 bias T1 bias(rest) hnA'(1-4) T2 T3
                # hnA'(5-8); paired copies (ACT: chunks 0,1,4,5 / DVE rest)
                hT_new = state.tile([128, KH, BL], F16, tag="hT")

                def tpose(c):
                    nc.tensor.transpose(p_tay[:, c * BL:(c + 1) * BL],
                                        hf_new[0:64, c * 128:(c + 1) * 128],
                                        ident16[0:64, :])
                    nc.tensor.transpose(p_tby[:, c * BL:(c + 1) * BL],
                                        hf_new[64:128, c * 128:(c + 1) * 128],
                                        ident16[64:128, :])

                last = t + 1 >= nsteps
                if not last:
                    nxt = emit_tiles_next()
                    bias_pair(nxt[0], AQ, brz, 0, HB, HQ, True, False)
                    bias_pair(nxt[3], ALL, brb2, 0, HB, HB, True, False)
                    bias_pair(nxt[1], NAs, bna, 0, NA, NA, True, False)
                    bias_pair(nxt[2], ALL, bnb2, 0, HB, HB, True, False)
                    bias_pair(nxt[4], AQ, brz, H, H + HB, HQ, True, False)
                    bias_pair(nxt[5], AQ, brz, H + HQ, H + HB + HQ, HQ, True,
                              False)
                tpose(0)
                tpose(1)
                nc.scalar.copy(hT_new[:, 0:2, :], p_tay[:, 0:2 * BL])
                nc.scalar.copy(hT_new[:, 4:6, :], p_tby[:, 0:2 * BL])
                if not last:
                    emit_hna(nxt[1], hT_new, 0, 4)
                tpose(2)
                tpose(3)
                nc.vector.tensor_copy(hT_new[:, 2:4, :], p_tay[:, 2 * BL:4 * BL])
                nc.vector.tensor_copy(hT_new[:, 6:8, :], p_tby[:, 2 * BL:4 * BL])
                if not last:
                    emit_hna(nxt[1], hT_new, 4, KH)
                    cur = nxt

                hf, hT = hf_new, hT_new

            # --- final y flush: y_(T-1) = x_(T-1) + h_T@W_tp.T + b_tp
            p_fl = ps_na.tile([128, NA], F32, tag="na")
            p_y = p_fl[:, 0:D // 2]
            for i, c in enumerate(HORD):
                nc.tensor.matmul(p_y[0:64, :], hT[:, c, :],
                                 wtp[:, c, 0:128], start=(i == 0), stop=False)
                nc.tensor.matmul(p_y[64:128, :], hT[:, c, :],
                                 wtp[:, c, 128:256], start=(i == 0), stop=False)
            nc.tensor.matmul(p_y[0:64, :], ones[:, 0:64], btp[:, 0:128],
                             start=False, stop=True)
            nc.tensor.matmul(p_y[64:128, :], ones[:, 64:128], btp[:, 128:256],
                             start=False, stop=True)
            y = ypool.tile([128, D // 2], F16, tag="y")
            nc.vector.tensor_add(y, xn, p_y)
            nc.sync.dma_start(out=Y_d[nsteps - 1, :, :], in_=y)

    nc.finalize()
    return nc


def _build_null():
    import concourse.mybir as mybir
    import concourse.tile as tile
    from concourse import bacc

    F32 = mybir.dt.float32
    F16 = mybir.dt.float16
    nc = bacc.Bacc(None, target_bir_lowering=False)
    hf0_d = nc.dram_tensor("hf0", [128, HB], F16, kind="ExternalInput")
    nc.dram_tensor("hT0", [128, KH, BL], F16, kind="ExternalInput")
    nc.dram_tensor("xf0", [128, D // 2], F16, kind="ExternalInput")
    nc.dram_tensor("xT0", [128, KD, BL], F16, kind="ExternalInput")
    nc.dram_tensor("whhT", [H, H3], F16, kind="ExternalInput")
    nc.dram_tensor("wihT", [D, H3], F16, kind="ExternalInput")
    nc.dram_tensor("wna0", [H, NA], F16, kind="ExternalInput")
    nc.dram_tensor("wna1", [H, NA], F16, kind="ExternalInput")
    nc.dram_tensor("wrb0", [D, HB], F16, kind="ExternalInput")
    nc.dram_tensor("wrb1", [D, HB], F16, kind="ExternalInput")
    nc.dram_tensor("wtpT", [H, D], F16, kind="ExternalInput")
    nc.dram_tensor("brz", [1, 2 * H], F16, kind="ExternalInput")
    nc.dram_tensor("brb", [1, 2 * HB], F16, kind="ExternalInput")
    nc.dram_tensor("bna", [1, 2 * NA], F16, kind="ExternalInput")
    nc.dram_tensor("bnb", [1, 2 * HB], F16, kind="ExternalInput")
    nc.dram_tensor("btp", [1, D], F16, kind="ExternalInput")
    nc.dram_tensor("ones", [1, 128], F16, kind="ExternalInput")
    Y_d = nc.dram_tensor("Y", [T, 128, D // 2], F16, kind="ExternalOutput")
    with tile.TileContext(nc) as tc:
        with tc.tile_pool(name="p", bufs=1) as p:
            tmp = p.tile([128, HB], F16)
            nc.sync.dma_start(out=tmp, in_=hf0_d[:, :])
            nc.sync.dma_start(out=Y_d[:, 0, 0:128], in_=tmp[0:128, 0:128])
    nc.finalize()
    return nc


def _get_nc(nsteps):
    if nsteps not in _CACHE:
        _CACHE[nsteps] = _build(nsteps)
    return _CACHE[nsteps]


def _fold(a):
    w = a.shape[1] // 2
    return np.concatenate([a[:, :w], a[:, w:]], axis=0)


def make_in_maps(h, gt, W_ih, W_hh, b_ih, b_hh, W_tp, b_tp):
    f32 = np.float32
    f16 = np.float16
    h = np.asarray(h, f32)
    x0 = np.ascontiguousarray(np.asarray(gt, f32)[:, 0, :])
    W_hh = np.asarray(W_hh, f32)
    W_ih = np.asarray(W_ih, f32)
    W_tp = np.asarray(W_tp, f32)
    whhT = np.ascontiguousarray(W_hh.T).astype(f16)
    wihT = np.ascontiguousarray(W_ih.T).astype(f16)
    wtpT = np.ascontiguousarray(W_tp.T).astype(f16)
    wna0 = np.ascontiguousarray(
        np.concatenate([whhT[:, 2 * H:2 * H + HQ], wtpT[:, 0:128]], axis=1))
    wna1 = np.ascontiguousarray(
        np.concatenate([whhT[:, 2 * H + HB:2 * H + HB + HQ],
                        wtpT[:, 128:256]], axis=1))
    # combined rB + in(q1,q2) x-side weights: [r cols | i_n cols]
    wrb0 = np.ascontiguousarray(
        np.concatenate([wihT[:, HQ:HB], wihT[:, 2 * H:2 * H + HQ]], axis=1))
    wrb1 = np.ascontiguousarray(
        np.concatenate([wihT[:, HB + HQ:H],
                        wihT[:, 2 * H + HB:2 * H + HB + HQ]], axis=1))
    b_sum = np.asarray(b_ih, f32) + np.asarray(b_hh, f32)
    brz = b_sum[None, :2 * H].astype(f16)
    b_hhn = np.asarray(b_hh, f32)[2 * H:]
    b_ihn = np.asarray(b_ih, f32)[2 * H:]
    b_tp = np.asarray(b_tp, f32)
    bna = np.concatenate([b_hhn[0:HQ], b_tp[0:128],
                          b_hhn[HB:HB + HQ], b_tp[128:256]])[None, :].astype(f16)
    bnb2 = np.concatenate([b_hhn[HQ:HB], b_ihn[HQ:HB],
                           b_hhn[HB + HQ:H],
                           b_ihn[HB + HQ:H]])[None, :].astype(f16)
    brb2 = np.concatenate([b_sum[HQ:HB], b_ihn[0:HQ],
                           b_sum[HB + HQ:H],
                           b_ihn[HB:HB + HQ]])[None, :].astype(f16)
    btp = np.ascontiguousarray(b_tp[None, :]).astype(f16)
    ones = np.ones((1, 128), f16)
    in_maps = []
    for c in range(NCORES):
        sl = slice(c * BL, (c + 1) * BL)
        hc = np.ascontiguousarray(h[sl])
        xc = np.ascontiguousarray(x0[sl])
        hTc = np.ascontiguousarray(hc.T.reshape(KH, 128, BL).transpose(1, 0, 2))
        xTc = np.ascontiguousarray(xc.T.reshape(KD, 128, BL).transpose(1, 0, 2))
        in_maps.append({
            "hf0": np.ascontiguousarray(_fold(hc)).astype(f16),
            "hT0": hTc.astype(f16),
            "xf0": np.ascontiguousarray(_fold(xc)).astype(f16),
            "xT0": xTc.astype(f16),
            "whhT": whhT, "wihT": wihT, "wna0": wna0, "wna1": wna1,
            "wrb0": wrb0, "wrb1": wrb1, "wtpT": wtpT,
            "brz": brz, "brb": brb2, "bna": bna, "bnb": bnb2, "btp": btp,
            "ones": ones,
        })
    return in_maps


def kernel(h, gt, W_ih, W_hh, b_ih, b_hh, W_tp, b_tp, time_steps):
    from concourse.bass_utils import run_bass_kernel_spmd
    nsteps = int(time_steps)
    assert nsteps == T, f"kernel hardcodes T={T}, got {nsteps}"
    nc = _get_nc(nsteps)
    in_maps = make_in_maps(h, gt, W_ih, W_hh, b_ih, b_hh, W_tp, b_tp)
    res = run_bass_kernel_spmd(nc, in_maps, core_ids=list(range(NCORES)),
                               trace=False)
    Y = np.concatenate([unfold_Y(res.results[c]["Y"]) for c in range(NCORES)],
                       axis=0)
    return Y.astype(np.float32)


def unfold_Y(Yd):
    """[T, 128, 128] -> [64, T, 256]"""
    out = np.empty((BL, T, D), np.float32)
    out[:, :, :D // 2] = Yd[:, 0:BL, :].transpose(1, 0, 2)
    out[:, :, D // 2:] = Yd[:, BL:128, :].transpose(1, 0, 2)
    return out


# revision 17
# speedup vs baseline: 9.4801x; 1.0056x over previous
"""GRU decoder kernel for Trainium2 (Bass/Tile), data-parallel over 8 cores.

Problem: nn_Decoder (B=512, T=128, D=256, H=1024), PyTorch GRUCell:
    gi = x @ W_ih.T + b_ih ; gh = h @ W_hh.T + b_hh
    r = sig(gi_r+gh_r); z = sig(gi_z+gh_z); n = tanh(gi_n + r*gh_n)
    h' = (1-z)*n + z*h ; y = x + h' @ W_tp.T + b_tp ; x' = y (x0=gt[:,0,:])

Design (batch 64/core, weights replicated; ~10.6us/step, ~1.37ms total,
rel_err ~9e-3 vs fp32 reference; v1 baseline was 1.50ms):
  - "Folded-H" layout: gate tiles [128, N] fp32 PSUM; partitions 0:64 =
    (batch, gate col group 0), 64:128 = (batch, group 1), produced
    concurrently by fp16 matmul pairs at tile_position (0,0)/(0,64)
    (full PE rate: N/2.4GHz per pair slot).
  - y-head FUSION: y_(t-1) = x_(t-1) + h_t@W_tp.T + b_tp rides as a
    128-col tail on the hn-A slots (N=384 rhs = [W_hh_n A-cols | W_tp]).
    The na bank gets no later writes, so DVE reads the y tail right
    after the 8 h-chunk slots with no same-bank write conflict.  This
    removes the v1 y-head (5 pair slots + LDW stalls) entirely.
  - SOFTWARE PIPELINE: step t+1's hnA slots run inside step t's endgame
    (around the h' transposes), so step t+1's r completes early and the
    ACT/DVE gate chain overlaps the z streams.  PE order per step:
      rA(8) yT rB(8) rAx rbx inBx hnB(8) zA(10) zB(10) |
      bias(6) T0 T1 hnA'(1-4) T2 T3 hnA'(5-8) | next step
  - PSUM banks (8): ra[256]=rA; rb[512]=rB+in(q1,q2) via a COMBINED
    N=512 x-matmul [W_ih_r | W_ih_n] so the bank's last write is a
    single stop that gates all its readers (the bank rule: an engine
    must never read a PSUM bank while the PE streams into any region
    of it); na[384]=hnA+y; nb[512]=inB written FIRST, then hnB whose
    stop gates t1B; za/zb[256]; ta/tb = fp16 transpose outputs.
  - Elementwise in fp16 intermediates (2x DVE rate; bf16 FAILS the
    2e-2 gate at ~3e-2, fp16 measures ~9e-3): sig/tanh halves on ACT,
    t1 halves / t2 quarters on DVE (t2 reads the three in-gate
    regions), d = h - n on the otherwise idle GPSIMD (SBUF-only
    operands; GPSIMD has no PSUM port), u/h' quartered on DVE so the
    post-zB tail is short (sig zB -> u_q -> h'_q per 128 cols).
  - Bias rows for t+1 pre-issued between zB and the transposes to fill
    the PE wait for h'; h' transposes are fp16 PE transpose-mode pairs,
    row groups 0/64 into DIFFERENT banks (same-bank pair crashes), with
    per-pair PSUM->SBUF copies (ACT: chunks 0,1,4,5 / DVE: 2,3,6,7)
    that unblock next-step hnA slots progressively.
  - y/x carry in fp16 (adds ~2e-3; output Y is fp16, cast on host).
  - Keeping PE micro-idles small matters: HAM re-throttles the PE clock
    to 1.2GHz (K=4/8) after idle windows; v2's ~1us endgame gap cost
    ~2x throttle penalties on the following streams.
"""
import numpy as np

B, T, D, H = 512, 128, 256, 1024
NCORES = 8
BL = B // NCORES  # 64
H3 = 3 * H
KH = H // 128  # 8
KD = D // 128  # 2
HB = 512       # folded column width (H/2)
HQ = 256       # half-group width
NA = HQ + 128  # hnA slot width (hn A-cols + y cols) = 384

_CACHE = {}


def _build(nsteps):
    import concourse.bass as bass
    import concourse.mybir as mybir
    import concourse.tile as tile
    from concourse import bacc
    from concourse.masks import make_identity

    F32 = mybir.dt.float32
    F16 = mybir.dt.float16
    AF = mybir.ActivationFunctionType

    nc = bacc.Bacc(None, target_bir_lowering=False)

    hf0_d = nc.dram_tensor("hf0", [128, HB], F16, kind="ExternalInput")
    hT0_d = nc.dram_tensor("hT0", [128, KH, BL], F16, kind="ExternalInput")
    xf0_d = nc.dram_tensor("xf0", [128, D // 2], F16, kind="ExternalInput")
    xT0_d = nc.dram_tensor("xT0", [128, KD, BL], F16, kind="ExternalInput")
    whh_d = nc.dram_tensor("whhT", [H, H3], F16, kind="ExternalInput")
    wih_d = nc.dram_tensor("wihT", [D, H3], F16, kind="ExternalInput")
    wna0_d = nc.dram_tensor("wna0", [H, NA], F16, kind="ExternalInput")
    wna1_d = nc.dram_tensor("wna1", [H, NA], F16, kind="ExternalInput")
    wrb0_d = nc.dram_tensor("wrb0", [D, HB], F16, kind="ExternalInput")
    wrb1_d = nc.dram_tensor("wrb1", [D, HB], F16, kind="ExternalInput")
    wtp_d = nc.dram_tensor("wtpT", [H, D], F16, kind="ExternalInput")
    brz_d = nc.dram_tensor("brz", [1, 2 * H], F16, kind="ExternalInput")
    brb_d = nc.dram_tensor("brb", [1, 2 * HB], F16, kind="ExternalInput")
    bna_d = nc.dram_tensor("bna", [1, 2 * NA], F16, kind="ExternalInput")
    bnb_d = nc.dram_tensor("bnb", [1, 2 * HB], F16, kind="ExternalInput")
    btp_d = nc.dram_tensor("btp", [1, D], F16, kind="ExternalInput")
    ones_d = nc.dram_tensor("ones", [1, 128], F16, kind="ExternalInput")
    Y_d = nc.dram_tensor("Y", [T, 128, D // 2], F16, kind="ExternalOutput")

    with tile.TileContext(nc) as tc:
        with (
            tc.tile_pool(name="wpool", bufs=1) as wpool,
            tc.tile_pool(name="state", bufs=2) as state,
            tc.tile_pool(name="gates", bufs=1) as gates,
            tc.tile_pool(name="ypool", bufs=3) as ypool,
            tc.tile_pool(name="ps_ra", bufs=1, space="PSUM") as ps_ra,
            tc.tile_pool(name="ps_na", bufs=1, space="PSUM") as ps_na,
            tc.tile_pool(name="ps_nb", bufs=1, space="PSUM") as ps_nb,
            tc.tile_pool(name="ps_rb", bufs=1, space="PSUM") as ps_rb,
            tc.tile_pool(name="ps_za", bufs=1, space="PSUM") as ps_za,
            tc.tile_pool(name="ps_zb", bufs=1, space="PSUM") as ps_zb,
            tc.tile_pool(name="ps_ta", bufs=1, space="PSUM") as ps_ta,
            tc.tile_pool(name="ps_tb", bufs=1, space="PSUM") as ps_tb,
        ):
            # --- resident weights
            whh = wpool.tile([128, KH, H3], F16)
            wih = wpool.tile([128, KD, H3], F16)
            wna0 = wpool.tile([128, KH, NA], F16)
            wna1 = wpool.tile([128, KH, NA], F16)
            wrb0 = wpool.tile([128, KD, HB], F16)
            wrb1 = wpool.tile([128, KD, HB], F16)
            wtp = wpool.tile([128, KH, D], F16)
            for c in range(KH):
                nc.sync.dma_start(out=whh[:, c, :], in_=whh_d[c * 128:(c + 1) * 128, :])
                nc.sync.dma_start(out=wna0[:, c, :], in_=wna0_d[c * 128:(c + 1) * 128, :])
                nc.sync.dma_start(out=wna1[:, c, :], in_=wna1_d[c * 128:(c + 1) * 128, :])
                nc.sync.dma_start(out=wtp[:, c, :], in_=wtp_d[c * 128:(c + 1) * 128, :])
            for c in range(KD):
                nc.sync.dma_start(out=wih[:, c, :], in_=wih_d[c * 128:(c + 1) * 128, :])
                nc.sync.dma_start(out=wrb0[:, c, :], in_=wrb0_d[c * 128:(c + 1) * 128, :])
                nc.sync.dma_start(out=wrb1[:, c, :], in_=wrb1_d[c * 128:(c + 1) * 128, :])
            brz = wpool.tile([1, 2 * H], F16)
            brb2 = wpool.tile([1, 2 * HB], F16)
            bna = wpool.tile([1, 2 * NA], F16)
            bnb2 = wpool.tile([1, 2 * HB], F16)
            btp = wpool.tile([1, D], F16)
            ones = wpool.tile([1, 128], F16)
            nc.sync.dma_start(out=brz, in_=brz_d[:, :])
            nc.sync.dma_start(out=brb2, in_=brb_d[:, :])
            nc.sync.dma_start(out=bna, in_=bna_d[:, :])
            nc.sync.dma_start(out=bnb2, in_=bnb_d[:, :])
            nc.sync.dma_start(out=btp, in_=btp_d[:, :])
            nc.sync.dma_start(out=ones, in_=ones_d[:, :])
            ident = wpool.tile([128, 64], F32)
            make_identity(nc, ident[0:64, :])
            make_identity(nc, ident[64:128, :])
            ident16 = wpool.tile([128, 64], F16)
            nc.vector.tensor_copy(ident16, ident)

            # --- state
            hf = state.tile([128, HB], F16, tag="hf")
            hT = state.tile([128, KH, BL], F16, tag="hT")
            xn = state.tile([128, D // 2], F16, tag="xn")
            xT = state.tile([128, KD, BL], F16, tag="xT")
            nc.sync.dma_start(out=hf, in_=hf0_d[:, :])
            nc.sync.dma_start(out=hT, in_=hT0_d[:, :, :])
            nc.sync.dma_start(out=xn, in_=xf0_d[:, :])
            nc.sync.dma_start(out=xT, in_=xT0_d[:, :, :])

            HORD = [0, 1, 4, 5, 2, 3, 6, 7]

            def pair(psum, cols, lhsT, rhs0, rhs1, start=False, stop=False):
                nc.tensor.matmul(psum[0:64, cols], lhsT, rhs0, start=start, stop=stop)
                nc.tensor.matmul(psum[64:128, cols], lhsT, rhs1, start=start, stop=stop)

            def bias_pair(psum, cols, bias_ap, b0, b1, width, start, stop):
                nc.tensor.matmul(psum[0:64, cols], ones[:, 0:64],
                                 bias_ap[:, b0:b0 + width], start=start, stop=stop)
                nc.tensor.matmul(psum[64:128, cols], ones[:, 64:128],
                                 bias_ap[:, b1:b1 + width], start=start, stop=stop)

            ALL = slice(0, HB)
            AQ = slice(0, HQ)
            NAs = slice(0, NA)

            def emit_tiles_next():
                p_ra = ps_ra.tile([128, HQ], F32, tag="ra")
                p_na = ps_na.tile([128, NA], F32, tag="na")
                p_nb = ps_nb.tile([128, HB], F32, tag="nb")
                p_rb = ps_rb.tile([128, HB], F32, tag="rb")
                p_za = ps_za.tile([128, HQ], F32, tag="za")
                p_zb = ps_zb.tile([128, HQ], F32, tag="zb")
                return p_ra, p_na, p_nb, p_rb, p_za, p_zb

            def emit_bias_all(cur):
                p_ra, p_na, p_nb, p_rb, p_za, p_zb = cur
                bias_pair(p_ra, AQ, brz, 0, HB, HQ, True, False)
                bias_pair(p_rb, ALL, brb2, 0, HB, HB, True, False)
                bias_pair(p_na, NAs, bna, 0, NA, NA, True, False)
                bias_pair(p_nb, ALL, bnb2, 0, HB, HB, True, False)
                bias_pair(p_za, AQ, brz, H, H + HB, HQ, True, False)
                bias_pair(p_zb, AQ, brz, H + HQ, H + HB + HQ, HQ, True, False)

            def emit_hna(p_na, hT_t, lo, hi):
                for i in range(lo, hi):
                    pair(p_na, NAs, hT_t[:, HORD[i], :], wna0[:, HORD[i], :],
                         wna1[:, HORD[i], :], stop=(i == KH - 1))

            # --- prologue: step-0 tiles, biases, hnA slots
            cur = emit_tiles_next()
            emit_bias_all(cur)
            emit_hna(cur[1], hT, 0, KH)

            for t in range(nsteps):
                p_ra, p_na, p_nb, p_rb, p_za, p_zb = cur
                # --- y_(t-1) extraction (DVE) and rA h-slots
                if t > 0:
                    y = ypool.tile([128, D // 2], F16, tag="y")
                    nc.vector.tensor_add(y, xn, p_na[:, HQ:NA])
                    nc.sync.dma_start(out=Y_d[t - 1, :, :], in_=y)
                for c in HORD:
                    pair(p_ra, AQ, hT[:, c, :], whh[:, c, 0:HQ],
                         whh[:, c, HB:HB + HQ])
                # --- yT transpose pair + xT copies (DVE)
                p_tay = ps_ta.tile([128, 5 * BL], F16, tag="ta")
                p_tby = ps_tb.tile([128, 5 * BL], F16, tag="tb")
                if t > 0:
                    nc.tensor.transpose(p_tay[:, 4 * BL:5 * BL], y[0:64, :],
                                        ident16[0:64, :])
                    nc.tensor.transpose(p_tby[:, 4 * BL:5 * BL], y[64:128, :],
                                        ident16[64:128, :])
                    xT_new = state.tile([128, KD, BL], F16, tag="xT")
                    nc.vector.tensor_copy(xT_new[:, 0, :], p_tay[:, 4 * BL:5 * BL])
                    nc.vector.tensor_copy(xT_new[:, 1, :], p_tby[:, 4 * BL:5 * BL])
                    xn, xT = y, xT_new
                # --- rB h-slots
                for c in HORD:
                    pair(p_rb, AQ, hT[:, c, :], whh[:, c, HQ:HB],
                         whh[:, c, HB + HQ:H])
                # --- rA x-slots (stop); combined rB+in x-slots (stop:
                # single final write per bank so every read dep covers all
                # bank writes)
                pair(p_ra, AQ, xT[:, 0, :], wih[:, 0, 0:HQ],
                     wih[:, 0, HB:HB + HQ])
                pair(p_ra, AQ, xT[:, 1, :], wih[:, 1, 0:HQ],
                     wih[:, 1, HB:HB + HQ], stop=True)
                pair(p_rb, ALL, xT[:, 0, :], wrb0[:, 0, :], wrb1[:, 0, :])
                pair(p_rb, ALL, xT[:, 1, :], wrb0[:, 1, :], wrb1[:, 1, :],
                     stop=True)
                # --- zA half-slots EARLY so sig(zA) -> u -> h'A
                # completes well before the zB stream ends
                def z_slots(p_zq, q):
                    z0 = H + HQ * q
                    z1 = H + HB + HQ * q
                    for c in HORD:
                        pair(p_zq, AQ, hT[:, c, :], whh[:, c, z0:z0 + HQ],
                             whh[:, c, z1:z1 + HQ])
                    pair(p_zq, AQ, xT[:, 0, :], wih[:, 0, z0:z0 + HQ],
                         wih[:, 0, z1:z1 + HQ])
                    pair(p_zq, AQ, xT[:, 1, :], wih[:, 1, z0:z0 + HQ],
                         wih[:, 1, z1:z1 + HQ], stop=True)

                # --- inB x-slots (nb-bank tail), then hnB h-slots: bank
                # fully written at the hnB stop, which gates t1B
                pair(p_nb, slice(HQ, HB), xT[:, 0, :],
                     wih[:, 0, 2 * H + HQ:2 * H + HB],
                     wih[:, 0, 2 * H + HB + HQ:H3])
                pair(p_nb, slice(HQ, HB), xT[:, 1, :],
                     wih[:, 1, 2 * H + HQ:2 * H + HB],
                     wih[:, 1, 2 * H + HB + HQ:H3], stop=True)
                for i, c in enumerate(HORD):
                    pair(p_nb, AQ, hT[:, c, :], whh[:, c, 2 * H + HQ:2 * H + HB],
                         whh[:, c, 2 * H + HB + HQ:H3], stop=(i == KH - 1))
                z_slots(p_za, 0)
                z_slots(p_zb, 1)

                # --- elementwise: fp16 intermediates; t1/sig/tanh halves,
                # t2 quartered (reads the three in-gate regions), d on
                # GPSIMD, u/h' quartered for a short post-zB tail.
                r_sb = gates.tile([128, HB], F16, tag="r")
                z_sb = gates.tile([128, HB], F16, tag="z")
                t1 = gates.tile([128, HB], F16, tag="t1")
                t2 = gates.tile([128, HB], F16, tag="t2")
                n_sb = gates.tile([128, HB], F16, tag="n")
                d_sb = gates.tile([128, HB], F16, tag="d")
                u_sb = gates.tile([128, HB], F16, tag="u")
                hf_new = state.tile([128, HB], F16, tag="hf")
                A = slice(0, HQ)
                Bc = slice(HQ, HB)
                Q = [slice(128 * q, 128 * (q + 1)) for q in range(4)]
                nc.scalar.activation(r_sb[:, A], p_ra[:, AQ], AF.Sigmoid)
                nc.scalar.activation(r_sb[:, Bc], p_rb[:, AQ], AF.Sigmoid)
                nc.vector.tensor_mul(t1[:, A], r_sb[:, A], p_na[:, AQ])
                nc.vector.tensor_mul(t1[:, Bc], r_sb[:, Bc], p_nb[:, AQ])
                nc.vector.tensor_add(t2[:, Q[0]], t1[:, Q[0]], p_rb[:, HQ:NA])
                nc.vector.tensor_add(t2[:, Q[1]], t1[:, Q[1]], p_rb[:, NA:HB])
                nc.vector.tensor_add(t2[:, Q[2]], t1[:, Q[2]], p_nb[:, HQ:NA])
                nc.vector.tensor_add(t2[:, Q[3]], t1[:, Q[3]], p_nb[:, NA:HB])
                nc.scalar.activation(n_sb[:, A], t2[:, A], AF.Tanh)
                nc.gpsimd.tensor_sub(d_sb[:, Q[0]], hf[:, Q[0]], n_sb[:, Q[0]])
                nc.gpsimd.tensor_sub(d_sb[:, Q[1]], hf[:, Q[1]], n_sb[:, Q[1]])
                nc.scalar.activation(z_sb[:, A], p_za[:, :], AF.Sigmoid)
                nc.scalar.activation(n_sb[:, Bc], t2[:, Bc], AF.Tanh)
                nc.gpsimd.tensor_sub(d_sb[:, Q[2]], hf[:, Q[2]], n_sb[:, Q[2]])
                nc.gpsimd.tensor_sub(d_sb[:, Q[3]], hf[:, Q[3]], n_sb[:, Q[3]])
                nc.scalar.activation(z_sb[:, Bc], p_zb[:, :], AF.Sigmoid)
                for q in range(4):
                    nc.vector.tensor_mul(u_sb[:, Q[q]], z_sb[:, Q[q]],
                                         d_sb[:, Q[q]])
                    nc.vector.tensor_add(hf_new[:, Q[q]], n_sb[:, Q[q]],
                                         u_sb[:, Q[q]])

                # --- endgame: T0 bias T1 bias(rest) hnA'(1-4) T2 T3
                # hnA'(5-8); paired copies (ACT: chunks 0,1,4,5 / DVE rest)
                hT_new = state.tile([128, KH, BL], F16, tag="hT")

                def tpose(c):
                    nc.tensor.transpose(p_tay[:, c * BL:(c + 1) * BL],
                                        hf_new[0:64, c * 128:(c + 1) * 128],
                                        ident16[0:64, :])
                    nc.tensor.transpose(p_tby[:, c * BL:(c + 1) * BL],
                                        hf_new[64:128, c * 128:(c + 1) * 128],
                                        ident16[64:128, :])

                last = t + 1 >= nsteps
                if not last:
                    nxt = emit_tiles_next()
                    bias_pair(nxt[0], AQ, brz, 0, HB, HQ, True, False)
                    bias_pair(nxt[3], ALL, brb2, 0, HB, HB, True, False)
                    bias_pair(nxt[1], NAs, bna, 0, NA, NA, True, False)
                    bias_pair(nxt[2], ALL, bnb2, 0, HB, HB, True, False)
                    bias_pair(nxt[4], AQ, brz, H, H + HB, HQ, True, False)
                    bias_pair(nxt[5], AQ, brz, H + HQ, H + HB + HQ, HQ, True,
                              False)
                tpose(0)
                tpose(1)
                nc.scalar.copy(hT_new[:, 0:2, :], p_tay[:, 0:2 * BL])
                nc.scalar.copy(hT_new[:, 4:6, :], p_tby[:, 0:2 * BL])
                if not last:
                    emit_hna(nxt[1], hT_new, 0, 4)
                tpose(2)
                tpose(3)
                nc.vector.tensor_copy(hT_new[:, 2:4, :], p_tay[:, 2 * BL:4 * BL])
                nc.vector.tensor_copy(hT_new[:, 6:8, :], p_tby[:, 2 * BL:4 * BL])
                if not last:
                    emit_hna(nxt[1], hT_new, 4, KH)
                    cur = nxt

                hf, hT = hf_new, hT_new

            # --- final y flush: y_(T-1) = x_(T-1) + h_T@W_tp.T + b_tp
            p_fl = ps_na.tile([128, NA], F32, tag="na")
            p_y = p_fl[:, 0:D // 2]
            for i, c in enumerate(HORD):
                nc.tensor.matmul(p_y[0:64, :], hT[:, c, :],
                                 wtp[:, c, 0:128], start=(i == 0), stop=False)
                nc.tensor.matmul(p_y[64:128, :], hT[:, c, :],
                                 wtp[:, c, 128:256], start=(i == 0), stop=False)
            nc.tensor.matmul(p_y[0:64, :], ones[:, 0:64], btp[:, 0:128],
                             start=False, stop=True)
            nc.tensor.matmul(p_y[64:128, :], ones[:, 64:128], btp[:, 128:256],
                             start=False, stop=True)
            y = ypool.tile([128, D // 2], F16, tag="y")
            nc.vector.tensor_add(y, xn, p_y)
            nc.sync.dma_start(out=Y_d[nsteps - 1, :, :], in_=y)

    nc.finalize()
    return nc


def _build_null():
    import concourse.mybir as mybir
    import concourse.tile as tile
    from concourse import bacc

    F32 = mybir.dt.float32
    F16 = mybir.dt.float16
    nc = bacc.Bacc(None, target_bir_lowering=False)
    hf0_d = nc.dram_tensor("hf0", [128, HB], F16, kind="ExternalInput")
    nc.dram_tensor("hT0", [128, KH, BL], F16, kind="ExternalInput")
    nc.dram_tensor("xf0", [128, D // 2], F16, kind="ExternalInput")
    nc.dram_tensor("xT0", [128, KD, BL], F16, kind="ExternalInput")
    nc.dram_tensor("whhT", [H, H3], F16, kind="ExternalInput")
    nc.dram_tensor("wihT", [D, H3], F16, kind="ExternalInput")
    nc.dram_tensor("wna0", [H, NA], F16, kind="ExternalInput")
    nc.dram_tensor("wna1", [H, NA], F16, kind="ExternalInput")
    nc.dram_tensor("wrb0", [D, HB], F16, kind="ExternalInput")
    nc.dram_tensor("wrb1", [D, HB], F16, kind="ExternalInput")
    nc.dram_tensor("wtpT", [H, D], F16, kind="ExternalInput")
    nc.dram_tensor("brz", [1, 2 * H], F16, kind="ExternalInput")
    nc.dram_tensor("brb", [1, 2 * HB], F16, kind="ExternalInput")
    nc.dram_tensor("bna", [1, 2 * NA], F16, kind="ExternalInput")
    nc.dram_tensor("bnb", [1, 2 * HB], F16, kind="ExternalInput")
    nc.dram_tensor("btp", [1, D], F16, kind="ExternalInput")
    nc.dram_tensor("ones", [1, 128], F16, kind="ExternalInput")
    Y_d = nc.dram_tensor("Y", [T, 128, D // 2], F16, kind="ExternalOutput")
    with tile.TileContext(nc) as tc:
        with tc.tile_pool(name="p", bufs=1) as p:
            tmp = p.tile([128, HB], F16)
            nc.sync.dma_start(out=tmp, in_=hf0_d[:, :])
            nc.sync.dma_start(out=Y_d[:, 0, 0:128], in_=tmp[0:128, 0:128])
    nc.finalize()
    return nc


def _get_nc(nsteps):
    if nsteps not in _CACHE:
        _CACHE[nsteps] = _build(nsteps)
    return _CACHE[nsteps]


def _fold(a):
    w = a.shape[1] // 2
    return np.concatenate([a[:, :w], a[:, w:]], axis=0)


def make_in_maps(h, gt, W_ih, W_hh, b_ih, b_hh, W_tp, b_tp):
    f32 = np.float32
    f16 = np.float16
    h = np.asarray(h, f32)
    x0 = np.ascontiguousarray(np.asarray(gt, f32)[:, 0, :])
    W_hh = np.asarray(W_hh, f32)
    W_ih = np.asarray(W_ih, f32)
    W_tp = np.asarray(W_tp, f32)
    whhT = np.ascontiguousarray(W_hh.T).astype(f16)
    wihT = np.ascontiguousarray(W_ih.T).astype(f16)
    wtpT = np.ascontiguousarray(W_tp.T).astype(f16)
    wna0 = np.ascontiguousarray(
        np.concatenate([whhT[:, 2 * H:2 * H + HQ], wtpT[:, 0:128]], axis=1))
    wna1 = np.ascontiguousarray(
        np.concatenate([whhT[:, 2 * H + HB:2 * H + HB + HQ],
                        wtpT[:, 128:256]], axis=1))
    # combined rB + in(q1,q2) x-side weights: [r cols | i_n cols]
    wrb0 = np.ascontiguousarray(
        np.concatenate([wihT[:, HQ:HB], wihT[:, 2 * H:2 * H + HQ]], axis=1))
    wrb1 = np.ascontiguousarray(
        np.concatenate([wihT[:, HB + HQ:H],
                        wihT[:, 2 * H + HB:2 * H + HB + HQ]], axis=1))
    b_sum = np.asarray(b_ih, f32) + np.asarray(b_hh, f32)
    brz = b_sum[None, :2 * H].astype(f16)
    b_hhn = np.asarray(b_hh, f32)[2 * H:]
    b_ihn = np.asarray(b_ih, f32)[2 * H:]
    b_tp = np.asarray(b_tp, f32)
    bna = np.concatenate([b_hhn[0:HQ], b_tp[0:128],
                          b_hhn[HB:HB + HQ], b_tp[128:256]])[None, :].astype(f16)
    bnb2 = np.concatenate([b_hhn[HQ:HB], b_ihn[HQ:HB],
                           b_hhn[HB + HQ:H],
                           b_ihn[HB + HQ:H]])[None, :].astype(f16)
    brb2 = np.concatenate([b_sum[HQ:HB], b_ihn[0:HQ],
                           b_sum[HB + HQ:H],
                           b_ihn[HB:HB + HQ]])[None, :].astype(f16)
    btp = np.ascontiguousarray(b_tp[None, :]).astype(f16)
    ones = np.ones((1, 128), f16)
    in_maps = []
    for c in range(NCORES):
        sl = slice(c * BL, (c + 1) * BL)
        hc = np.ascontiguousarray(h[sl])
        xc = np.ascontiguousarray(x0[sl])
        hTc = np.ascontiguousarray(hc.T.reshape(KH, 128, BL).transpose(1, 0, 2))
        xTc = np.ascontiguousarray(xc.T.reshape(KD, 128, BL).transpose(1, 0, 2))
        in_maps.append({
            "hf0": np.ascontiguousarray(_fold(hc)).astype(f16),
            "hT0": hTc.astype(f16),
            "xf0": np.ascontiguousarray(_fold(xc)).astype(f16),
            "xT0": xTc.astype(f16),
            "whhT": whhT, "wihT": wihT, "wna0": wna0, "wna1": wna1,
            "wrb0": wrb0, "wrb1": wrb1, "wtpT": wtpT,
            "brz": brz, "brb": brb2, "bna": bna, "bnb": bnb2, "btp": btp,
            "ones": ones,
        })
    return in_maps


def kernel(h, gt, W_ih, W_hh, b_ih, b_hh, W_tp, b_tp, time_steps):
    from concourse.bass_utils import run_bass_kernel_spmd
    nsteps = int(time_steps)
    assert nsteps == T, f"kernel hardcodes T={T}, got {nsteps}"
    nc = _get_nc(nsteps)
    in_maps = make_in_maps(h, gt, W_ih, W_hh, b_ih, b_hh, W_tp, b_tp)
    res = run_bass_kernel_spmd(nc, in_maps, core_ids=list(range(NCORES)),
                               trace=False)
    Y = np.concatenate([unfold_Y(res.results[c]["Y"]) for c in range(NCORES)],
                       axis=0)
    return Y.astype(np.float32)


def unfold_Y(Yd):
    """[T, 128, 128] -> [64, T, 256]"""
    out = np.empty((BL, T, D), np.float32)
    out[:, :, :D // 2] = Yd[:, 0:BL, :].transpose(1, 0, 2)
    out[:, :, D // 2:] = Yd[:, BL:128, :].transpose(1, 0, 2)
    return out
